# revision 20
# baseline (speedup 1.0000x reference)
"""Trainium2 Bass kernel for nn_EnhancedDFSMN (self-contained).

Sharding: data parallel over batch -> 8 NeuronCores x 4 sequences each.
Per core each sequence [1500, 257] is processed in 128-token chunks
(12 chunks, zero-padded to 1536 tokens).

Restructurings (validated against the JAX reference to ~5e-7 absmax):
 - DFSMN tap matrices collapse to per-tap scalars (row sums); then
   h + ctx_mem + ctx_lookahead == B @ h with B banded Toeplitz (special
   warm-up block for t < memory_size).  Done as 3 PE matmuls per chunk
   with host-precomputed band blocks.
 - Activations live tokens-on-partitions; matmul inputs are produced
   features-on-partitions via PE transposes (identity matmul).
 - Matmul biases folded in via ones-row on lhsT + bias row on weights;
   per-partition (feature) biases folded into ACT passes.
 - Softmax without max subtraction (logits bounded).
 - Notch product accumulated as prod(e_i - 1) (NF=4 even); protected
   region handled by memset of the first 26 columns.
 - LayerNorm via bn_stats/bn_aggr + one ACT scale/bias pass.
Matmuls run in float32r.
"""

import numpy as np
from contextlib import ExitStack

import concourse.bass as bass
import concourse.bacc as bacc
import concourse.tile as tile
from concourse import mybir
from concourse.masks import make_identity

F32 = mybir.dt.float32
F32R = mybir.dt.float32r
AF = mybir.ActivationFunctionType
ALU = mybir.AluOpType

IN_DIM = 257
HID = [512, 384, 256, 128]
MEM = 50
LA = 5
NF = 4
EPS = 1e-5
P = 128

N_CORES = 8
B_FULL = 32
B_CORE = B_FULL // N_CORES
T = 1500

_RUN_CACHE = {}


# ----------------------------------------------------------------------------
# host-side precompute
# ----------------------------------------------------------------------------

def _band_blocks(wm, wl):
    """lhsT blocks ([s, t]) for out[t,:] = sum_s B[t,s] h[s,:]."""
    CH = P
    s_idx = np.arange(CH)[:, None]
    t_idx = np.arange(CH)[None, :]
    self_blk = np.zeros((CH, CH), np.float32)
    d = s_idx - t_idx
    m = (d >= -MEM) & (d <= -1)
    self_blk[m] = wm[(d + MEM)[m]]
    self_blk[d == 0] = 1.0
    m = (d >= 1) & (d <= LA)
    self_blk[m] = wl[(d - 1)[m]]
    self0 = self_blk.copy()
    for t in range(MEM):
        col = np.zeros(CH, np.float32)
        col[:t] = wm[:t]
        col[t] = 1.0
        hi = min(CH, t + LA + 1)
        col[t + 1:hi] = wl[:hi - t - 1]
        self0[:, t] = col
    prev = np.zeros((CH, CH), np.float32)
    d = (s_idx - CH) - t_idx
    m = (d >= -MEM) & (d <= -1)
    prev[m] = wm[(d + MEM)[m]]
    nxt = np.zeros((CH, CH), np.float32)
    d = (s_idx + CH) - t_idx
    m = (d >= 1) & (d <= LA)
    nxt[m] = wl[(d - 1)[m]]
    return self0, self_blk, prev, nxt


def _prep_host(p):
    f32 = np.float32
    d = {}

    def cat_bias(w, b, pad=False):
        r = np.concatenate([np.asarray(w, f32), np.asarray(b, f32)[None]], 0)
        if pad:
            r = np.concatenate([r, np.zeros((r.shape[0], 1), f32)], 1)
        return r

    d['fa_w1'] = np.asarray(p['fa_w1'], f32)
    d['fg_w1'] = np.asarray(p['fg_w1'], f32)
    d['fa_b1m'] = np.asarray(p['fa_b1'], f32).reshape(4, P).T.copy()
    d['fa_b1m2'] = 2.0 * d['fa_b1m']
    d['fg_b1m'] = np.asarray(p['fg_b1'], f32).reshape(4, P).T.copy()
    d['fa_w2p'] = cat_bias(p['fa_w2'], p['fa_b2'], pad=True)
    d['fg_w2p'] = cat_bias(p['fg_w2'], p['fg_b2'], pad=True)
    d['nf_w1'] = np.asarray(p['nf_w1'], f32)
    d['nf_b1c'] = np.asarray(p['nf_b1'], f32).reshape(P, 1)
    d['nf_w2p'] = cat_bias(p['nf_w2'], p['nf_b2'])
    d['bd_w1'] = np.asarray(p['bd_w1'], f32)
    d['bd_b1c'] = np.asarray(p['bd_b1'], f32).reshape(P, 1)
    d['bd_w2p'] = cat_bias(p['bd_w2'], p['bd_b2'], pad=True)

    # din=257 weights, layout [258, dout]:
    #   rows 0..255 = w[:256]; row 256/257 chosen to match the lhsT pair.
    # skip: lhsT pair = xhi rows (x256, ones) -> rhs rows (w256, bias)
    for i in range(4):
        w = np.asarray(p[f's{i}_w'], f32)
        b = np.asarray(p[f's{i}_b'], f32) + np.asarray(p[f'l{i}_be'], f32)
        d[f'sw{i}'] = np.concatenate([w[:256], w[256:257], b[None]], 0)
    # l0: lhsT pair = ffnhi rows (ffn256, ones) -> rhs rows (w256, bias)
    w = np.asarray(p['l0_w'], f32)
    b = np.asarray(p['l0_b'], f32)
    d['lw0'] = np.concatenate([w[:256], w[256:257], b[None]], 0)
    d['lw1'] = cat_bias(p['l1_w'], p['l1_b'])
    d['lw2'] = cat_bias(p['l2_w'], p['l2_b'])
    d['lw3'] = cat_bias(p['l3_w'], p['l3_b'])
    d['outwp'] = cat_bias(p['out_w'], p['out_b'], pad=True)
    d['gatewp'] = cat_bias(p['gate_w'], p['gate_b'], pad=True)

    for i in range(4):
        wm = np.asarray(p[f'l{i}_mem'], f32).sum(-1)
        wl = np.asarray(p[f'l{i}_la'], f32).sum(-1)
        s0, sg, pv, nx = _band_blocks(wm, wl)
        d[f'bself0_{i}'] = s0
        d[f'bself_{i}'] = sg
        d[f'bprev_{i}'] = pv
        d[f'bnext_{i}'] = nx

    d['fidx'] = np.linspace(0.0, 1.0, IN_DIM).astype(f32)
    d['onesrow'] = np.ones((1, ((T + P - 1) // P) * P), f32)
    flags = {
        'fa2_b': bool(np.any(p['fa_b2'])), 'fg2_b': bool(np.any(p['fg_b2'])),
        'nf2_b': bool(np.any(p['nf_b2'])), 'bd2_b': bool(np.any(p['bd_b2'])),
        'l1_b': bool(np.any(p['l1_b'])), 'l2_b': bool(np.any(p['l2_b'])),
        'l3_b': bool(np.any(p['l3_b'])),
        'out_b': bool(np.any(p['out_b'])), 'gate_b': bool(np.any(p['gate_b'])),
        'has_g': any(bool(np.any(np.asarray(p[f'l{i}_g']) != 1.0))
                     for i in range(4)),
    }
    d['flags'] = flags
    if flags['has_g']:
        for i in range(4):
            d[f'g_bcast{i}'] = np.broadcast_to(
                np.asarray(p[f'l{i}_g'], f32), (P, HID[i])).copy()
    return d


# ----------------------------------------------------------------------------
# device program
# ----------------------------------------------------------------------------

def _r(ap):
    return ap.bitcast(F32R)


def build_program(n_seq, t_len, flags):
    has_g = flags['has_g']
    nch = (t_len + P - 1) // P
    nc2 = (nch + 1) // 2
    TL = nch * P

    nc = bacc.Bacc("TRN2", target_bir_lowering=False, debug=False,
                   num_devices=N_CORES)

    F32_INPUTS = {'x', 'fa_b1m', 'fg_b1m', 'fa_b1m2', 'nf_b1c', 'bd_b1c', 'fidx'} | \
        {f'g_bcast{i}' for i in range(4)}

    def din(name, shape):
        dt = F32 if name in F32_INPUTS else F32R
        return nc.dram_tensor(name, list(shape), dt, kind="ExternalInput").ap()

    x_d = din("x", (n_seq, t_len, IN_DIM))
    out0_d = nc.dram_tensor("out0", [n_seq, t_len, IN_DIM], F32,
                            kind="ExternalOutput").ap()
    out1_d = nc.dram_tensor("out1", [n_seq, t_len, IN_DIM], F32,
                            kind="ExternalOutput").ap()

    shapes = [
        ('fa_w1', (257, 512)), ('fg_w1', (257, 512)),
        ('fa_b1m', (P, 4)), ('fg_b1m', (P, 4)), ('fa_b1m2', (P, 4)),
        ('fa_w2p', (513, 258)), ('fg_w2p', (513, 258)),
        ('nf_w1', (257, 128)), ('nf_b1c', (P, 1)), ('nf_w2p', (129, 8)),
        ('bd_w1', (257, 128)), ('bd_b1c', (P, 1)), ('bd_w2p', (129, 258)),
        ('sw0', (258, 512)), ('sw1', (258, 384)), ('sw2', (258, 256)),
        ('sw3', (258, 128)),
        ('lw0', (258, 512)), ('lw1', (513, 384)), ('lw2', (385, 256)),
        ('lw3', (257, 128)),
        ('outwp', (129, 258)), ('gatewp', (129, 258)),
        ('fidx', (IN_DIM,)),
        ('onesrow', (1, TL)),
    ]
    w_d = {nm: din(nm, shp) for nm, shp in shapes}
    for i in range(4):
        for nm in (f'bself0_{i}', f'bself_{i}', f'bprev_{i}', f'bnext_{i}'):
            w_d[nm] = din(nm, (P, P))
        if has_g:
            w_d[f'g_bcast{i}'] = din(f'g_bcast{i}', (P, HID[i]))

    with ExitStack() as ctx:
        tc = ctx.enter_context(tile.TileContext(nc))
        consts = ctx.enter_context(tc.tile_pool(name="consts", bufs=1))
        seqp = ctx.enter_context(tc.tile_pool(name="seqp", bufs=1))
        work = ctx.enter_context(tc.tile_pool(name="work", bufs=1))
        small = ctx.enter_context(tc.tile_pool(name="small", bufs=3))
        psum = ctx.enter_context(tc.tile_pool(name="psum", bufs=1,
                                              space="PSUM"))

        cw = {}

        def ctile(name, src_ap):
            t = consts.tile(list(src_ap.shape), src_ap.dtype, name=name)
            nc.sync.dma_start(out=t, in_=src_ap)
            cw[name] = t
            return t

        for nm in ('fa_w1', 'fg_w1'):
            ctile(nm + "_k0", w_d[nm][0:128, :])
            ctile(nm + "_k1", w_d[nm][128:256, :])
            ctile(nm + "_k2", w_d[nm][256:257, :])
        ctile("fa_b1m", w_d['fa_b1m'])
        ctile("fa_b1m2", w_d['fa_b1m2'])
        ctile("fg_b1m", w_d['fg_b1m'])
        for nm in ('fa_w2p', 'fg_w2p'):
            for k in range(4):
                ctile(f"{nm}_k{k}", w_d[nm][k * 128:(k + 1) * 128, :])
        if flags['fa2_b']:
            ctile("fa_w2p_b", w_d['fa_w2p'][512:513, :])
        if flags['fg2_b']:
            ctile("fg_w2p_b", w_d['fg_w2p'][512:513, :])
        for nm in ('nf_w1', 'bd_w1'):
            ctile(nm + "_k0", w_d[nm][0:128, :])
            ctile(nm + "_k1", w_d[nm][128:256, :])
            ctile(nm + "_k2", w_d[nm][256:257, :])
        ctile("nf_b1c", w_d['nf_b1c'])
        ctile("bd_b1c", w_d['bd_b1c'])
        ctile("nf_w2p_k0", w_d['nf_w2p'][0:128, :])
        if flags['nf2_b']:
            ctile("nf_w2p_b", w_d['nf_w2p'][128:129, :])
        ctile("bd_w2p_k0", w_d['bd_w2p'][0:128, :])
        if flags['bd2_b']:
            ctile("bd_w2p_b", w_d['bd_w2p'][128:129, :])
        for i in range(4):
            ctile(f"sw{i}_k0", w_d[f'sw{i}'][0:128, :])
            ctile(f"sw{i}_k1", w_d[f'sw{i}'][128:256, :])
            ctile(f"sw{i}_k2", w_d[f'sw{i}'][256:258, :])
        ctile("lw0_k0", w_d['lw0'][0:128, :])
        ctile("lw0_k1", w_d['lw0'][128:256, :])
        ctile("lw0_k2", w_d['lw0'][256:258, :])
        for k in range(4):
            ctile(f"lw1_k{k}", w_d['lw1'][k * 128:(k + 1) * 128, :])
        if flags['l1_b']:
            ctile("lw1_b", w_d['lw1'][512:513, :])
        for k in range(3):
            ctile(f"lw2_k{k}", w_d['lw2'][k * 128:(k + 1) * 128, :])
        if flags['l2_b']:
            ctile("lw2_b", w_d['lw2'][384:385, :])
        for k in range(2):
            ctile(f"lw3_k{k}", w_d['lw3'][k * 128:(k + 1) * 128, :])
        if flags['l3_b']:
            ctile("lw3_b", w_d['lw3'][256:257, :])
        ctile("outwp_k0", w_d['outwp'][0:128, :])
        if flags['out_b']:
            ctile("outwp_b", w_d['outwp'][128:129, :])
        ctile("gatewp_k0", w_d['gatewp'][0:128, :])
        if flags['gate_b']:
            ctile("gatewp_b", w_d['gatewp'][128:129, :])
        for i in range(4):
            for nm in (f'bself0_{i}', f'bself_{i}', f'bprev_{i}', f'bnext_{i}'):
                ctile(nm, w_d[nm])
            if has_g:
                ctile(f'g_bcast{i}', w_d[f'g_bcast{i}'])

        fidx_b = consts.tile([P, IN_DIM], F32, name="fidx_b")
        fidx_bc = bass.AP(tensor=w_d['fidx'].tensor, offset=w_d['fidx'].offset,
                          ap=[[0, P]] + list(w_d['fidx'].ap))
        nc.gpsimd.dma_start(out=fidx_b, in_=fidx_bc)

        identity = consts.tile([P, P], F32, name="identity")
        make_identity(nc, identity)
        eps_t = consts.tile([P, 1], F32, name="eps_t")
        nc.vector.memset(eps_t, EPS)

        n_prot = int(np.sum(np.linspace(0.0, 1.0, IN_DIM) <= 0.1))

        for s in range(n_seq):
            xT = seqp.tile([P, 2, TL], F32R, tag="xT", name="xT")
            xhi = seqp.tile([2, TL], F32R, tag="xhi")      # (x256, ones)
            ffnhi = seqp.tile([2, TL], F32R, tag="ffnhi")  # (ffn256, ones)
            h_t = seqp.tile([P, nch, 512], F32R, tag="h_t")
            fT = seqp.tile([P, 4, TL], F32R, tag="fT", name="fT")
            f3a = seqp.tile([P, nch, 128], F32, tag="f3a", name="f3a")
            any_bias = any(flags[k] for k in
                           ('fa2_b', 'fg2_b', 'nf2_b', 'bd2_b', 'l1_b',
                            'l2_b', 'l3_b', 'out_b', 'gate_b'))
            if any_bias:
                ones_r = seqp.tile([1, TL], F32R, tag="ones_r")
                nc.gpsimd.memset(ones_r.bitcast(F32), 1.0)
            nc.sync.dma_start(out=xhi[1:2, :], in_=w_d['onesrow'])
            nc.sync.dma_start(out=ffnhi[1:2, :], in_=w_d['onesrow'])

            # ================= phase A =================
            for c2 in range(nc2):
                cols = bass.ts(c2, 2 * P)
                x_t = work.tile([P, 2, IN_DIM], F32, tag="x_t", bufs=2)
                for sub in range(2):
                    c = 2 * c2 + sub
                    rows = min(P, t_len - c * P)
                    if rows < P:
                        zb = (rows // 32) * 32
                        nc.vector.memset(x_t[zb:P, sub, :], 0.0)
                    nc.sync.dma_start(out=x_t[:rows, sub, :],
                                      in_=x_d[s, c * P: c * P + rows, :])
                    ccols = bass.ts(c, P)
                    ps_tr = psum.tile([P, 2, P], F32, tag="tr", bufs=2)
                    for blk in range(2):
                        nc.tensor.transpose(
                            ps_tr[:, blk, :],
                            x_t[:, sub, blk * P:(blk + 1) * P], identity)
                    nc.scalar.copy(out=xT[:, :, ccols], in_=ps_tr)
                    ps_t1 = psum.tile([P, 2, P], F32, tag="tr", bufs=2)
                    nc.tensor.transpose(ps_t1[0:1, 0, :],
                                        x_t[:, sub, 256:257], identity)
                    nc.scalar.copy(out=xhi[0:1, ccols],
                                   in_=ps_t1[0:1, 0, :])

                # ---- table state: sigmoid_and_others ----
                t1T = work.tile([P, 4, 2 * P], F32R, tag="t1T")
                g1T = work.tile([P, 4, 2 * P], F32R, tag="g1T")
                tu = work.tile([P, 4, 2 * P], F32, tag="tu")
                for w1, bm1, dst, fn in (
                        ('fa_w1', 'fa_b1m', t1T, AF.Tanh),
                        ('fg_w1', 'fg_b1m', g1T, AF.Relu)):
                    for m in range(4):
                        mc = bass.ts(m, P)
                        ps = psum.tile([P, 512], F32, tag="mmA", bufs=3)
                        psl = ps[:, :2 * P]
                        nc.tensor.matmul(psl, _r(cw[w1 + '_k0'][:, mc]),
                                         xT[:, 0, cols], start=True,
                                         stop=False)
                        nc.tensor.matmul(psl, _r(cw[w1 + '_k1'][:, mc]),
                                         xT[:, 1, cols], start=False,
                                         stop=False)
                        nc.tensor.matmul(psl, _r(cw[w1 + '_k2'][:, mc]),
                                         xhi[0:1, cols], start=False,
                                         stop=True)
                        if fn == AF.Tanh:
                            # tanh(z) = 1 - 2/(exp(2z)+1); per-part bias b:
                            # exp(2z+2b) via scale=2, bias=2b folded on host
                            nc.scalar.activation(tu[:, m, :], psl, AF.Exp,
                                                 scale=2.0,
                                                 bias=cw['fa_b1m2'][:, m:m + 1])
                            nc.vector.tensor_scalar_add(tu[:, m, :],
                                                        tu[:, m, :], 1.0)
                            nc.vector.reciprocal_approx_fast(tu[:, m, :],
                                                             tu[:, m, :])
                            nc.vector.tensor_scalar(dst[:, m, :], tu[:, m, :],
                                                    -2.0, 1.0, op0=ALU.mult,
                                                    op1=ALU.add)
                        else:
                            nc.scalar.activation(dst[:, m, :], psl, fn,
                                                 bias=cw[bm1][:, m:m + 1])

                # gate = sigmoid(fg2)
                gate = work.tile([P, 2, IN_DIM], F32, tag="gate", bufs=2)
                for sub in range(2):
                    scs = bass.ts(2 * c2 + sub, P)
                    ps = psum.tile([P, 512], F32, tag="mmB", bufs=3)
                    psl = ps[:, :IN_DIM + 1]
                    for m in range(4):
                        nc.tensor.matmul(
                            psl, _r(g1T[:, m, sub * P:(sub + 1) * P]),
                            _r(cw['fg_w2p_k' + str(m)]), start=(m == 0),
                            stop=(m == 3 and not flags['fg2_b']))
                    if flags['fg2_b']:
                        nc.tensor.matmul(psl, _r(ones_r[0:1, scs]),
                                         _r(cw['fg_w2p_b']), start=False,
                                         stop=True)
                    nc.scalar.activation(gate[:, sub, :], ps[:, :IN_DIM],
                                         AF.Exp, scale=-1.0)
                    nc.vector.tensor_scalar_add(gate[:, sub, :],
                                                gate[:, sub, :], 1.0)
                    nc.vector.reciprocal_approx_fast(gate[:, sub, :],
                                                     gate[:, sub, :])

                # bd hidden + bp (sigmoid)
                bdh = work.tile([P, 2 * P], F32R, tag="bdh", bufs=2)
                ps = psum.tile([P, 512], F32, tag="mmA", bufs=3)
                psl = ps[:, :2 * P]
                nc.tensor.matmul(psl, _r(cw['bd_w1_k0']), xT[:, 0, cols],
                                 start=True, stop=False)
                nc.tensor.matmul(psl, _r(cw['bd_w1_k1']), xT[:, 1, cols],
                                 start=False, stop=False)
                nc.tensor.matmul(psl, _r(cw['bd_w1_k2']), xhi[0:1, cols],
                                 start=False, stop=True)
                nc.scalar.activation(bdh, psl, AF.Relu, bias=cw['bd_b1c'])

                bp_c = work.tile([P, 2, IN_DIM], F32, tag="bp_c", bufs=2)
                for sub in range(2):
                    c = 2 * c2 + sub
                    rows = min(P, t_len - c * P)
                    ps = psum.tile([P, 512], F32, tag="mmB", bufs=3)
                    psl = ps[:, :IN_DIM + 1]
                    nc.tensor.matmul(psl, _r(bdh[:, sub * P:(sub + 1) * P]),
                                     _r(cw['bd_w2p_k0']), start=True,
                                     stop=not flags['bd2_b'])
                    if flags['bd2_b']:
                        nc.tensor.matmul(psl, _r(ones_r[0:1, bass.ts(c, P)]),
                                         _r(cw['bd_w2p_b']), start=False,
                                         stop=True)
                    nc.scalar.activation(bp_c[:, sub, :], ps[:, :IN_DIM],
                                         AF.Exp, scale=-1.0)
                    nc.vector.tensor_scalar_add(bp_c[:, sub, :],
                                                bp_c[:, sub, :], 1.0)
                    nc.vector.reciprocal_approx_fast(bp_c[:, sub, :],
                                                     bp_c[:, sub, :])
                    nc.sync.dma_start(out=out1_d[s, c * P:c * P + rows, :],
                                      in_=bp_c[:rows, sub, :])

                # ---- table state: natural_log_exp_and_others ----
                attn = work.tile([P, 2, IN_DIM], F32, tag="attn")
                ssum = small.tile([P, 2], F32, tag="ssum")
                for sub in range(2):
                    scs = bass.ts(2 * c2 + sub, P)
                    ps = psum.tile([P, 512], F32, tag="mmB", bufs=3)
                    psl = ps[:, :IN_DIM + 1]
                    for m in range(4):
                        nc.tensor.matmul(
                            psl, _r(t1T[:, m, sub * P:(sub + 1) * P]),
                            _r(cw['fa_w2p_k' + str(m)]), start=(m == 0),
                            stop=(m == 3 and not flags['fa2_b']))
                    if flags['fa2_b']:
                        nc.tensor.matmul(psl, _r(ones_r[0:1, scs]),
                                         _r(cw['fa_w2p_b']), start=False,
                                         stop=True)
                    nc.scalar.activation(attn[:, sub, :], ps[:, :IN_DIM],
                                         AF.Exp,
                                         accum_out=ssum[:, sub:sub + 1])
                nc.vector.reciprocal(ssum, ssum)
                for sub in range(2):
                    nc.vector.tensor_scalar(attn[:, sub, :], attn[:, sub, :],
                                            ssum[:, sub:sub + 1], None,
                                            op0=ALU.mult)
                ff = work.tile([P, 2, IN_DIM], F32, tag="ff", bufs=2)
                nc.vector.tensor_tensor(ff, x_t, attn, op=ALU.mult)
                nc.vector.tensor_tensor(ff, ff, gate, op=ALU.mult)

                # ffT (pre-notch)
                ffT = work.tile([P, 2, 2 * P], F32R, tag="ffT", bufs=2)
                ffT_hi = work.tile([1, 2 * P], F32R, tag="ffT_hi", bufs=2)
                for sub in range(2):
                    scol = bass.ts(sub, P)
                    ps_tr = psum.tile([P, 2, P], F32, tag="tr", bufs=2)
                    for blk in range(2):
                        nc.tensor.transpose(
                            ps_tr[:, blk, :],
                            ff[:, sub, blk * P:(blk + 1) * P], identity)
                    nc.vector.tensor_copy(out=ffT[:, :, scol], in_=ps_tr)
                    ps_t1 = psum.tile([P, 2, P], F32, tag="tr", bufs=2)
                    nc.tensor.transpose(ps_t1[0:1, 0, :], ff[:, sub, 256:257],
                                        identity)
                    nc.vector.tensor_copy(out=ffT_hi[:, scol],
                                          in_=ps_t1[0:1, 0, :])

                nfh = work.tile([P, 2 * P], F32R, tag="nfh", bufs=2)
                ps = psum.tile([P, 512], F32, tag="mmA", bufs=3)
                psl = ps[:, :2 * P]
                nc.tensor.matmul(psl, _r(cw['nf_w1_k0']), ffT[:, 0, :],
                                 start=True, stop=False)
                nc.tensor.matmul(psl, _r(cw['nf_w1_k1']), ffT[:, 1, :],
                                 start=False, stop=False)
                nc.tensor.matmul(psl, _r(cw['nf_w1_k2']), ffT_hi,
                                 start=False, stop=True)
                nc.scalar.activation(nfh, psl, AF.Relu, bias=cw['nf_b1c'])

                # notch params + response
                r_t = work.tile([P, 2, IN_DIM], F32, tag="r_t", bufs=2)
                for sub in range(2):
                    ps = psum.tile([P, 512], F32, tag="mmB", bufs=3)
                    psl = ps[:, :8]
                    nc.tensor.matmul(psl, _r(nfh[:, sub * P:(sub + 1) * P]),
                                     _r(cw['nf_w2p_k0']), start=True,
                                     stop=not flags['nf2_b'])
                    if flags['nf2_b']:
                        nc.tensor.matmul(
                            psl, _r(ones_r[0:1, bass.ts(2 * c2 + sub, P)]),
                            _r(cw['nf_w2p_b']), start=False, stop=True)
                    negc = small.tile([P, 4], F32, tag="negc")
                    negk = small.tile([P, 4], F32, tag="negk")
                    nc.scalar.activation(negc, psl[:, 0:4], AF.Identity,
                                         scale=-1.0)
                    nc.scalar.activation(negk, psl[:, 4:8], AF.Exp)
                    nc.scalar.activation(negk, negk, AF.Ln, bias=1.0)
                    nc.vector.tensor_tensor(negk, negk, negk, op=ALU.mult)
                    nc.vector.reciprocal(negk, negk)
                    nc.vector.tensor_scalar(negk, negk,
                                            -1.0 / (2.0 * 1.3 * 1.3), None,
                                            op0=ALU.mult)
                    for i in range(NF):
                        u_t = work.tile([P, IN_DIM], F32, tag="u_t", bufs=2)
                        nc.scalar.activation(u_t, fidx_b, AF.Square,
                                             bias=negc[:, i:i + 1])
                        nc.scalar.activation(u_t, u_t, AF.Exp,
                                             scale=negk[:, i:i + 1])
                        if i == 0:
                            nc.vector.tensor_scalar(r_t[:, sub, :], u_t, 1.0,
                                                    None, op0=ALU.subtract)
                        else:
                            nc.vector.scalar_tensor_tensor(
                                r_t[:, sub, :], u_t, 1.0, r_t[:, sub, :],
                                op0=ALU.subtract, op1=ALU.mult)
                nc.gpsimd.memset(r_t[:, :, 0:n_prot], 1.0)
                nc.vector.tensor_tensor(ff, ff, r_t, op=ALU.mult)

                # ffn transposed into fT[:, 0:2] and ffnhi
                for sub in range(2):
                    c = 2 * c2 + sub
                    ccols = bass.ts(c, P)
                    ps_tr = psum.tile([P, 2, P], F32, tag="tr", bufs=2)
                    for blk in range(2):
                        nc.tensor.transpose(
                            ps_tr[:, blk, :],
                            ff[:, sub, blk * P:(blk + 1) * P], identity)
                    nc.vector.tensor_copy(out=fT[:, 0:2, ccols], in_=ps_tr)
                    ps_t1 = psum.tile([P, 2, P], F32, tag="tr", bufs=2)
                    nc.tensor.transpose(ps_t1[0:1, 0, :], ff[:, sub, 256:257],
                                        identity)
                    nc.vector.tensor_copy(out=ffnhi[0:1, ccols],
                                          in_=ps_t1[0:1, 0, :])

            # ================= DFSMN layers =================
            # table state: sqrt_and_friends (Sqrt/Identity/Copy/Relu/Square)
            for li in range(4):
                dout = HID[li]
                mt = dout // P
                for c in range(nch):
                    ccols = bass.ts(c, P)
                    ps = psum.tile([P, 512], F32, tag="mmA", bufs=3)
                    psl = ps[:, :dout]
                    if li == 0:
                        nc.tensor.matmul(psl, fT[:, 0, ccols],
                                         _r(cw['lw0_k0']), start=True,
                                         stop=False)
                        nc.tensor.matmul(psl, fT[:, 1, ccols],
                                         _r(cw['lw0_k1']), start=False,
                                         stop=False)
                        nc.tensor.matmul(psl, _r(ffnhi[:, ccols]),
                                         _r(cw['lw0_k2']), start=False,
                                         stop=True)
                    else:
                        nk = HID[li - 1] // P
                        bias_f = flags[f'l{li}_b']
                        for k in range(nk):
                            nc.tensor.matmul(
                                psl, fT[:, k, ccols],
                                _r(cw[f'lw{li}_k{k}']), start=(k == 0),
                                stop=(k == nk - 1 and not bias_f))
                        if bias_f:
                            nc.tensor.matmul(psl, _r(ones_r[0:1, ccols]),
                                             _r(cw[f'lw{li}_b']), start=False,
                                             stop=True)
                    rows_c = min(P, t_len - c * P)
                    if rows_c < P:
                        zb = (rows_c // 32) * 32
                        nc.vector.memset(h_t[zb:P, c, :dout].bitcast(F32), 0.0)
                    nc.scalar.copy(out=h_t[:rows_c, c, :dout],
                                   in_=psl[:rows_c, :])

                for c in range(nch):
                    ccols = bass.ts(c, P)
                    ps_c = psum.tile([P, 512], F32, tag="mmB", bufs=3)
                    pcl = ps_c[:, :dout]
                    sblk = cw[f'bself0_{li}'] if c == 0 else cw[f'bself_{li}']
                    last = (c == nch - 1)
                    nc.tensor.matmul(pcl, _r(sblk), h_t[:, c, :dout],
                                     start=True, stop=(c == 0 and last))
                    if c > 0:
                        nc.tensor.matmul(pcl, _r(cw[f'bprev_{li}']),
                                         h_t[:, c - 1, :dout],
                                         start=False, stop=last)
                    if not last:
                        nc.tensor.matmul(pcl, _r(cw[f'bnext_{li}']),
                                         h_t[:, c + 1, :dout],
                                         start=False, stop=True)
                    st6 = small.tile([P, 6], F32, tag="st6")
                    mv = small.tile([P, 2], F32, tag="mv")
                    nc.vector.bn_stats(st6, pcl)
                    nc.vector.bn_aggr(mv, st6)
                    rstd = small.tile([P, 1], F32, tag="rstd")
                    nmr = small.tile([P, 1], F32, tag="nmr")
                    nc.scalar.activation(rstd, mv[:, 1:2], AF.Sqrt, bias=eps_t)
                    nc.vector.reciprocal(rstd, rstd)
                    nc.vector.scalar_tensor_tensor(nmr, mv[:, 0:1], -1.0, rstd,
                                                   op0=ALU.mult, op1=ALU.mult)
                    ps_s = psum.tile([P, 512], F32, tag="mmA", bufs=3)
                    pss = ps_s[:, :dout]
                    nc.tensor.matmul(pss, xT[:, 0, ccols],
                                     _r(cw[f'sw{li}_k0']), start=True,
                                     stop=False)
                    nc.tensor.matmul(pss, xT[:, 1, ccols],
                                     _r(cw[f'sw{li}_k1']), start=False,
                                     stop=False)
                    nc.tensor.matmul(pss, _r(xhi[:, ccols]),
                                     _r(cw[f'sw{li}_k2']), start=False,
                                     stop=True)
                    ln_t = work.tile([P, 512], F32, tag="ln_t", bufs=2)
                    lnl = ln_t[:, :dout]
                    nc.scalar.activation(lnl, pcl, AF.Identity, scale=rstd,
                                         bias=nmr)
                    if has_g:
                        nc.vector.tensor_tensor(lnl, lnl, cw[f'g_bcast{li}'],
                                                op=ALU.mult)
                    nc.vector.tensor_tensor(lnl, lnl, pss, op=ALU.add)
                    if li < 3:
                        nc.scalar.activation(lnl, lnl, AF.Relu)
                        ps_tr = psum.tile([P, mt, P], F32, tag="tr", bufs=2,
                                          name="ps_tr")
                        for m in range(mt):
                            nc.tensor.transpose(
                                ps_tr[:, m, :], lnl[:, m * P:(m + 1) * P],
                                identity)
                        nc.vector.tensor_copy(out=fT[:, :mt, ccols],
                                              in_=ps_tr)
                    else:
                        nc.scalar.activation(f3a[:, c, :], lnl, AF.Relu)

            # ================= output stage =================
            for c in range(nch):
                ccols = bass.ts(c, P)
                rows = min(P, t_len - c * P)
                ps_tr = psum.tile([P, 2, P], F32, tag="tr", bufs=2)
                nc.tensor.transpose(ps_tr[:, 0, :], f3a[:, c, :], identity)
                f3T = work.tile([P, P], F32R, tag="f3T", bufs=2)
                nc.vector.tensor_copy(out=f3T, in_=ps_tr[:, 0, :])

                ps_o = psum.tile([P, 512], F32, tag="mmA", bufs=3)
                pso = ps_o[:, :IN_DIM + 1]
                nc.tensor.matmul(pso, f3T, _r(cw['outwp_k0']),
                                 start=True, stop=not flags['out_b'])
                if flags['out_b']:
                    nc.tensor.matmul(pso, _r(ones_r[0:1, ccols]),
                                     _r(cw['outwp_b']), start=False,
                                     stop=True)
                ps_g = psum.tile([P, 512], F32, tag="mmB", bufs=3)
                psg = ps_g[:, :IN_DIM + 1]
                nc.tensor.matmul(psg, f3T, _r(cw['gatewp_k0']),
                                 start=True, stop=not flags['gate_b'])
                if flags['gate_b']:
                    nc.tensor.matmul(psg, _r(ones_r[0:1, ccols]),
                                     _r(cw['gatewp_b']), start=False,
                                     stop=True)
                og = work.tile([P, IN_DIM], F32, tag="og", bufs=2)
                nc.scalar.activation(og, ps_g[:, :IN_DIM], AF.Exp, scale=-1.0)
                nc.vector.tensor_scalar_add(og, og, 1.0)
                nc.vector.reciprocal_approx_fast(og, og)
                xb = work.tile([P, IN_DIM], F32, tag="xb", bufs=2)
                bpb = work.tile([P, IN_DIM], F32, tag="bpb", bufs=2)
                nc.sync.dma_start(out=xb[:rows, :],
                                  in_=x_d[s, c * P:c * P + rows, :])
                nc.sync.dma_start(out=bpb[:rows, :],
                                  in_=out1_d[s, c * P:c * P + rows, :])
                bm = work.tile([P, IN_DIM], mybir.dt.uint8, tag="bm", bufs=2)
                nc.vector.tensor_scalar(bm[:rows, :], bpb[:rows, :],
                                        0.6, None, op0=ALU.is_gt)
                res = work.tile([P, IN_DIM], F32, tag="res", bufs=2)
                nc.vector.tensor_copy(out=res[:rows, :],
                                      in_=ps_o[:rows, :IN_DIM])
                nc.vector.copy_predicated(res[:rows, :], bm[:rows, :],
                                          xb[:rows, :])
                nc.vector.tensor_tensor(res[:rows, :], res[:rows, :],
                                        og[:rows, :], op=ALU.mult)
                nc.sync.dma_start(out=out0_d[s, c * P:c * P + rows, :],
                                  in_=res[:rows, :])
    nc.compile()
    return nc


# ----------------------------------------------------------------------------
# entry point
# ----------------------------------------------------------------------------

def _build_cached(n_seq, t_len, flags):
    key = (n_seq, t_len, tuple(sorted(flags.items())))
    if key not in _RUN_CACHE:
        _RUN_CACHE[key] = build_program(n_seq, t_len, flags)
    return _RUN_CACHE[key]


def kernel(x, params, **run_kwargs):
    from concourse.bass_utils import run_bass_kernel_spmd

    x = np.ascontiguousarray(np.asarray(x, np.float32))
    host = _prep_host(params)
    flags = host['flags']
    nc = _build_cached(B_CORE, T, flags)

    w_map = {k: np.ascontiguousarray(v) for k, v in host.items()
             if isinstance(v, np.ndarray)}
    in_maps = []
    for core in range(N_CORES):
        m = dict(w_map)
        m['x'] = x[core * B_CORE:(core + 1) * B_CORE]
        in_maps.append(m)

    res = run_bass_kernel_spmd(nc, in_maps, core_ids=list(range(N_CORES)),
                               **run_kwargs)
    out0 = np.concatenate([r['out0'] for r in res.results], 0)
    out1 = np.concatenate([r['out1'] for r in res.results], 0)
    return out0, out1


# revision 22
# speedup vs baseline: 1.1332x; 1.1332x over previous
"""Trainium2 Bass kernel for nn_EnhancedDFSMN (self-contained).

Sharding: data parallel over batch -> 8 NeuronCores x 4 sequences each.
Per core each sequence [1500, 257] is processed in 128-token chunks
(12 chunks, zero-padded to 1536 tokens).

Restructurings (validated against the JAX reference to ~5e-7 absmax):
 - DFSMN tap matrices collapse to per-tap scalars (row sums); then
   h + ctx_mem + ctx_lookahead == B @ h with B banded Toeplitz (special
   warm-up block for t < memory_size).  Done as 3 PE matmuls per chunk
   with host-precomputed band blocks.
 - Activations live tokens-on-partitions; matmul inputs are produced
   features-on-partitions via PE transposes (identity matmul).
 - Matmul biases folded in via ones-row on lhsT + bias row on weights;
   per-partition (feature) biases folded into ACT passes.
 - Softmax without max subtraction (logits bounded).
 - Notch product accumulated as prod(e_i - 1) (NF=4 even); protected
   region handled by memset of the first 26 columns.
 - LayerNorm via bn_stats/bn_aggr + one ACT scale/bias pass.
Matmuls run in float32r.
"""

import numpy as np
from contextlib import ExitStack

import concourse.bass as bass
import concourse.bacc as bacc
import concourse.tile as tile
from concourse import mybir
from concourse.masks import make_identity

F32 = mybir.dt.float32
F32R = mybir.dt.float32r
AF = mybir.ActivationFunctionType
ALU = mybir.AluOpType

IN_DIM = 257
HID = [512, 384, 256, 128]
MEM = 50
LA = 5
NF = 4
EPS = 1e-5
P = 128

N_CORES = 8
B_FULL = 32
B_CORE = B_FULL // N_CORES
T = 1500

_RUN_CACHE = {}


# ----------------------------------------------------------------------------
# host-side precompute
# ----------------------------------------------------------------------------

def _band_blocks(wm, wl):
    """lhsT blocks ([s, t]) for out[t,:] = sum_s B[t,s] h[s,:]."""
    CH = P
    s_idx = np.arange(CH)[:, None]
    t_idx = np.arange(CH)[None, :]
    self_blk = np.zeros((CH, CH), np.float32)
    d = s_idx - t_idx
    m = (d >= -MEM) & (d <= -1)
    self_blk[m] = wm[(d + MEM)[m]]
    self_blk[d == 0] = 1.0
    m = (d >= 1) & (d <= LA)
    self_blk[m] = wl[(d - 1)[m]]
    self0 = self_blk.copy()
    for t in range(MEM):
        col = np.zeros(CH, np.float32)
        col[:t] = wm[:t]
        col[t] = 1.0
        hi = min(CH, t + LA + 1)
        col[t + 1:hi] = wl[:hi - t - 1]
        self0[:, t] = col
    prev = np.zeros((CH, CH), np.float32)
    d = (s_idx - CH) - t_idx
    m = (d >= -MEM) & (d <= -1)
    prev[m] = wm[(d + MEM)[m]]
    nxt = np.zeros((CH, CH), np.float32)
    d = (s_idx + CH) - t_idx
    m = (d >= 1) & (d <= LA)
    nxt[m] = wl[(d - 1)[m]]
    return self0, self_blk, prev, nxt


def _prep_host(p):
    f32 = np.float32
    d = {}

    def cat_bias(w, b, pad=False):
        r = np.concatenate([np.asarray(w, f32), np.asarray(b, f32)[None]], 0)
        if pad:
            r = np.concatenate([r, np.zeros((r.shape[0], 1), f32)], 1)
        return r

    d['fa_w1'] = np.asarray(p['fa_w1'], f32)
    d['fg_w1'] = np.asarray(p['fg_w1'], f32)
    d['fa_b1m'] = np.asarray(p['fa_b1'], f32).reshape(4, P).T.copy()
    d['fa_b1m2'] = 2.0 * d['fa_b1m']
    d['fg_b1m'] = np.asarray(p['fg_b1'], f32).reshape(4, P).T.copy()
    d['fa_w2p'] = cat_bias(p['fa_w2'], p['fa_b2'], pad=True)
    d['fg_w2p'] = cat_bias(p['fg_w2'], p['fg_b2'], pad=True)
    d['nf_w1'] = np.asarray(p['nf_w1'], f32)
    d['nf_b1c'] = np.asarray(p['nf_b1'], f32).reshape(P, 1)
    d['nf_w2p'] = cat_bias(p['nf_w2'], p['nf_b2'])
    d['bd_w1'] = np.asarray(p['bd_w1'], f32)
    d['bd_b1c'] = np.asarray(p['bd_b1'], f32).reshape(P, 1)
    d['bd_w2p'] = cat_bias(p['bd_w2'], p['bd_b2'], pad=True)

    # din=257 weights, layout [258, dout]:
    #   rows 0..255 = w[:256]; row 256/257 chosen to match the lhsT pair.
    # skip: lhsT pair = xhi rows (x256, ones) -> rhs rows (w256, bias)
    for i in range(4):
        w = np.asarray(p[f's{i}_w'], f32)
        b = np.asarray(p[f's{i}_b'], f32) + np.asarray(p[f'l{i}_be'], f32)
        d[f'sw{i}'] = np.concatenate([w[:256], w[256:257], b[None]], 0)
    # l0: lhsT pair = ffnhi rows (ffn256, ones) -> rhs rows (w256, bias)
    w = np.asarray(p['l0_w'], f32)
    b = np.asarray(p['l0_b'], f32)
    d['lw0'] = np.concatenate([w[:256], w[256:257], b[None]], 0)
    d['lw1'] = cat_bias(p['l1_w'], p['l1_b'])
    d['lw2'] = cat_bias(p['l2_w'], p['l2_b'])
    d['lw3'] = cat_bias(p['l3_w'], p['l3_b'])
    d['outwp'] = cat_bias(p['out_w'], p['out_b'], pad=True)
    d['gatewp'] = cat_bias(p['gate_w'], p['gate_b'], pad=True)

    for i in range(4):
        wm = np.asarray(p[f'l{i}_mem'], f32).sum(-1)
        wl = np.asarray(p[f'l{i}_la'], f32).sum(-1)
        s0, sg, pv, nx = _band_blocks(wm, wl)
        d[f'bself0_{i}'] = s0
        d[f'bself_{i}'] = sg
        d[f'bprev_{i}'] = pv
        d[f'bnext_{i}'] = nx

    d['fidx'] = np.linspace(0.0, 1.0, IN_DIM).astype(f32)
    d['onesrow'] = np.ones((1, ((T + P - 1) // P) * P), f32)
    flags = {
        'fa2_b': bool(np.any(p['fa_b2'])), 'fg2_b': bool(np.any(p['fg_b2'])),
        'nf2_b': bool(np.any(p['nf_b2'])), 'bd2_b': bool(np.any(p['bd_b2'])),
        'l1_b': bool(np.any(p['l1_b'])), 'l2_b': bool(np.any(p['l2_b'])),
        'l3_b': bool(np.any(p['l3_b'])),
        'out_b': bool(np.any(p['out_b'])), 'gate_b': bool(np.any(p['gate_b'])),
        'has_g': any(bool(np.any(np.asarray(p[f'l{i}_g']) != 1.0))
                     for i in range(4)),
    }
    d['flags'] = flags
    if flags['has_g']:
        for i in range(4):
            d[f'g_bcast{i}'] = np.broadcast_to(
                np.asarray(p[f'l{i}_g'], f32), (P, HID[i])).copy()
    return d


# ----------------------------------------------------------------------------
# device program
# ----------------------------------------------------------------------------

def _r(ap):
    return ap.bitcast(F32R)


def build_program(n_seq, t_len, flags):
    has_g = flags['has_g']
    nch = (t_len + P - 1) // P
    nc2 = (nch + 1) // 2
    TL = nch * P

    nc = bacc.Bacc("TRN2", target_bir_lowering=False, debug=False,
                   num_devices=N_CORES)

    F32_INPUTS = {'x', 'fa_b1m', 'fg_b1m', 'fa_b1m2', 'nf_b1c', 'bd_b1c', 'fidx'} | \
        {f'g_bcast{i}' for i in range(4)}

    def din(name, shape):
        dt = F32 if name in F32_INPUTS else F32R
        return nc.dram_tensor(name, list(shape), dt, kind="ExternalInput").ap()

    x_d = din("x", (n_seq, t_len, IN_DIM))
    out0_d = nc.dram_tensor("out0", [n_seq, t_len, IN_DIM], F32,
                            kind="ExternalOutput").ap()
    out1_d = nc.dram_tensor("out1", [n_seq, t_len, IN_DIM], F32,
                            kind="ExternalOutput").ap()

    shapes = [
        ('fa_w1', (257, 512)), ('fg_w1', (257, 512)),
        ('fa_b1m', (P, 4)), ('fg_b1m', (P, 4)), ('fa_b1m2', (P, 4)),
        ('fa_w2p', (513, 258)), ('fg_w2p', (513, 258)),
        ('nf_w1', (257, 128)), ('nf_b1c', (P, 1)), ('nf_w2p', (129, 8)),
        ('bd_w1', (257, 128)), ('bd_b1c', (P, 1)), ('bd_w2p', (129, 258)),
        ('sw0', (258, 512)), ('sw1', (258, 384)), ('sw2', (258, 256)),
        ('sw3', (258, 128)),
        ('lw0', (258, 512)), ('lw1', (513, 384)), ('lw2', (385, 256)),
        ('lw3', (257, 128)),
        ('outwp', (129, 258)), ('gatewp', (129, 258)),
        ('fidx', (IN_DIM,)),
        ('onesrow', (1, TL)),
    ]
    w_d = {nm: din(nm, shp) for nm, shp in shapes}
    for i in range(4):
        for nm in (f'bself0_{i}', f'bself_{i}', f'bprev_{i}', f'bnext_{i}'):
            w_d[nm] = din(nm, (P, P))
        if has_g:
            w_d[f'g_bcast{i}'] = din(f'g_bcast{i}', (P, HID[i]))

    with ExitStack() as ctx:
        tc = ctx.enter_context(tile.TileContext(nc))
        consts = ctx.enter_context(tc.tile_pool(name="consts", bufs=1))
        seqp = ctx.enter_context(tc.tile_pool(name="seqp", bufs=1))
        work = ctx.enter_context(tc.tile_pool(name="work", bufs=1))
        small = ctx.enter_context(tc.tile_pool(name="small", bufs=3))
        psum = ctx.enter_context(tc.tile_pool(name="psum", bufs=1,
                                              space="PSUM"))

        cw = {}

        def ctile(name, src_ap):
            t = consts.tile(list(src_ap.shape), src_ap.dtype, name=name)
            nc.sync.dma_start(out=t, in_=src_ap)
            cw[name] = t
            return t

        for nm in ('fa_w1', 'fg_w1'):
            ctile(nm + "_k0", w_d[nm][0:128, :])
            ctile(nm + "_k1", w_d[nm][128:256, :])
            ctile(nm + "_k2", w_d[nm][256:257, :])
        ctile("fa_b1m", w_d['fa_b1m'])
        ctile("fa_b1m2", w_d['fa_b1m2'])
        ctile("fg_b1m", w_d['fg_b1m'])
        for nm in ('fa_w2p', 'fg_w2p'):
            for k in range(4):
                ctile(f"{nm}_k{k}", w_d[nm][k * 128:(k + 1) * 128, :])
        if flags['fa2_b']:
            ctile("fa_w2p_b", w_d['fa_w2p'][512:513, :])
        if flags['fg2_b']:
            ctile("fg_w2p_b", w_d['fg_w2p'][512:513, :])
        for nm in ('nf_w1', 'bd_w1'):
            ctile(nm + "_k0", w_d[nm][0:128, :])
            ctile(nm + "_k1", w_d[nm][128:256, :])
            ctile(nm + "_k2", w_d[nm][256:257, :])
        ctile("nf_b1c", w_d['nf_b1c'])
        ctile("bd_b1c", w_d['bd_b1c'])
        ctile("nf_w2p_k0", w_d['nf_w2p'][0:128, :])
        if flags['nf2_b']:
            ctile("nf_w2p_b", w_d['nf_w2p'][128:129, :])
        ctile("bd_w2p_k0", w_d['bd_w2p'][0:128, :])
        if flags['bd2_b']:
            ctile("bd_w2p_b", w_d['bd_w2p'][128:129, :])
        for i in range(4):
            ctile(f"sw{i}_k0", w_d[f'sw{i}'][0:128, :])
            ctile(f"sw{i}_k1", w_d[f'sw{i}'][128:256, :])
            ctile(f"sw{i}_k2", w_d[f'sw{i}'][256:258, :])
        ctile("lw0_k0", w_d['lw0'][0:128, :])
        ctile("lw0_k1", w_d['lw0'][128:256, :])
        ctile("lw0_k2", w_d['lw0'][256:258, :])
        for k in range(4):
            ctile(f"lw1_k{k}", w_d['lw1'][k * 128:(k + 1) * 128, :])
        if flags['l1_b']:
            ctile("lw1_b", w_d['lw1'][512:513, :])
        for k in range(3):
            ctile(f"lw2_k{k}", w_d['lw2'][k * 128:(k + 1) * 128, :])
        if flags['l2_b']:
            ctile("lw2_b", w_d['lw2'][384:385, :])
        for k in range(2):
            ctile(f"lw3_k{k}", w_d['lw3'][k * 128:(k + 1) * 128, :])
        if flags['l3_b']:
            ctile("lw3_b", w_d['lw3'][256:257, :])
        ctile("outwp_k0", w_d['outwp'][0:128, :])
        if flags['out_b']:
            ctile("outwp_b", w_d['outwp'][128:129, :])
        ctile("gatewp_k0", w_d['gatewp'][0:128, :])
        if flags['gate_b']:
            ctile("gatewp_b", w_d['gatewp'][128:129, :])
        for i in range(4):
            for nm in (f'bself0_{i}', f'bself_{i}', f'bprev_{i}', f'bnext_{i}'):
                ctile(nm, w_d[nm])
            if has_g:
                ctile(f'g_bcast{i}', w_d[f'g_bcast{i}'])

        fidx_b = consts.tile([P, IN_DIM], F32, name="fidx_b")
        fidx_bc = bass.AP(tensor=w_d['fidx'].tensor, offset=w_d['fidx'].offset,
                          ap=[[0, P]] + list(w_d['fidx'].ap))
        nc.gpsimd.dma_start(out=fidx_b, in_=fidx_bc)

        identity = consts.tile([P, P], F32, name="identity")
        make_identity(nc, identity)
        eps_t = consts.tile([P, 1], F32, name="eps_t")
        nc.vector.memset(eps_t, EPS)

        n_prot = int(np.sum(np.linspace(0.0, 1.0, IN_DIM) <= 0.1))

        for s in range(n_seq):
            xT = seqp.tile([P, 2, TL], F32R, tag="xT", name="xT")
            xhi = seqp.tile([2, TL], F32R, tag="xhi")      # (x256, ones)
            ffnhi = seqp.tile([2, TL], F32R, tag="ffnhi")  # (ffn256, ones)
            h_t = seqp.tile([P, nch, 512], F32R, tag="h_t")
            fT = seqp.tile([P, 4, TL], F32R, tag="fT", name="fT")
            f3a = seqp.tile([P, nch, 128], F32, tag="f3a", name="f3a")
            any_bias = any(flags[k] for k in
                           ('fa2_b', 'fg2_b', 'nf2_b', 'bd2_b', 'l1_b',
                            'l2_b', 'l3_b', 'out_b', 'gate_b'))
            if any_bias:
                ones_r = seqp.tile([1, TL], F32R, tag="ones_r")
                nc.gpsimd.memset(ones_r.bitcast(F32), 1.0)
            nc.sync.dma_start(out=xhi[1:2, :], in_=w_d['onesrow'])
            nc.sync.dma_start(out=ffnhi[1:2, :], in_=w_d['onesrow'])

            # ================= phase A (2-stage software pipeline) =====
            # front(c2): x load/transpose + fa1/fg1 hidden + gate + burst
            # back(c2):  softmax + ff + notch chain + ffn transposes
            # Emitting front(c2+1) before back(c2) gives the PE independent
            # work to chew on while back(c2)'s serial ACT/DVE notch chain
            # runs (measured 16us/chunk PE stall otherwise).
            st = {}

            def front(c2):
                cols = bass.ts(c2, 2 * P)
                x_t = work.tile([P, 2, IN_DIM], F32, tag="x_t", bufs=2,
                                name="x_t")
                for sub in range(2):
                    c = 2 * c2 + sub
                    rows = min(P, t_len - c * P)
                    if rows < P:
                        zb = (rows // 32) * 32
                        nc.vector.memset(x_t[zb:P, sub, :], 0.0)
                    nc.sync.dma_start(out=x_t[:rows, sub, :],
                                      in_=x_d[s, c * P: c * P + rows, :])
                    ccols = bass.ts(c, P)
                    ps_tr = psum.tile([P, 2, P], F32, tag="tr", bufs=2,
                                      name="ps_tr")
                    for blk in range(2):
                        nc.tensor.transpose(
                            ps_tr[:, blk, :],
                            x_t[:, sub, blk * P:(blk + 1) * P], identity)
                    nc.scalar.copy(out=xT[:, :, ccols], in_=ps_tr)
                    ps_t1 = psum.tile([P, 2, P], F32, tag="tr", bufs=2,
                                      name="ps_t1")
                    nc.tensor.transpose(ps_t1[0:1, 0, :],
                                        x_t[:, sub, 256:257], identity)
                    nc.scalar.copy(out=xhi[0:1, ccols],
                                   in_=ps_t1[0:1, 0, :])

                t1T = work.tile([P, 4, 2 * P], F32R, tag="t1T", bufs=2,
                                name="t1T")
                g1T = work.tile([P, 4, 2 * P], F32R, tag="g1T", bufs=2,
                                name="g1T")
                tu = work.tile([P, 4, 2 * P], F32, tag="tu", name="tu")
                for w1, bm1, dst, fn in (
                        ('fa_w1', 'fa_b1m', t1T, AF.Tanh),
                        ('fg_w1', 'fg_b1m', g1T, AF.Relu)):
                    for m in range(4):
                        mc = bass.ts(m, P)
                        ps = psum.tile([P, 512], F32, tag="mmA", bufs=3,
                                       name="ps")
                        psl = ps[:, :2 * P]
                        nc.tensor.matmul(psl, _r(cw[w1 + '_k0'][:, mc]),
                                         xT[:, 0, cols], start=True,
                                         stop=False)
                        nc.tensor.matmul(psl, _r(cw[w1 + '_k1'][:, mc]),
                                         xT[:, 1, cols], start=False,
                                         stop=False)
                        nc.tensor.matmul(psl, _r(cw[w1 + '_k2'][:, mc]),
                                         xhi[0:1, cols], start=False,
                                         stop=True)
                        if fn == AF.Tanh:
                            nc.scalar.activation(tu[:, m, :], psl, AF.Exp,
                                                 scale=2.0,
                                                 bias=cw['fa_b1m2'][:, m:m + 1])
                            nc.vector.tensor_scalar_add(tu[:, m, :],
                                                        tu[:, m, :], 1.0)
                            nc.vector.reciprocal_approx_fast(tu[:, m, :],
                                                             tu[:, m, :])
                            nc.vector.tensor_scalar(dst[:, m, :], tu[:, m, :],
                                                    -2.0, 1.0, op0=ALU.mult,
                                                    op1=ALU.add)
                        else:
                            nc.scalar.activation(dst[:, m, :], psl, fn,
                                                 bias=cw[bm1][:, m:m + 1])

                gate = work.tile([P, 2, IN_DIM], F32, tag="gate", bufs=2,
                                 name="gate")
                for sub in range(2):
                    scs = bass.ts(2 * c2 + sub, P)
                    ps = psum.tile([P, 512], F32, tag="mmB", bufs=3,
                                   name="ps")
                    psl = ps[:, :IN_DIM + 1]
                    for m in range(4):
                        nc.tensor.matmul(
                            psl, _r(g1T[:, m, sub * P:(sub + 1) * P]),
                            _r(cw['fg_w2p_k' + str(m)]), start=(m == 0),
                            stop=(m == 3 and not flags['fg2_b']))
                    if flags['fg2_b']:
                        nc.tensor.matmul(psl, _r(ones_r[0:1, scs]),
                                         _r(cw['fg_w2p_b']), start=False,
                                         stop=True)
                    nc.scalar.activation(gate[:, sub, :], ps[:, :IN_DIM],
                                         AF.Exp, scale=-1.0)
                    nc.vector.tensor_scalar_add(gate[:, sub, :],
                                                gate[:, sub, :], 1.0)
                    nc.vector.reciprocal_approx_fast(gate[:, sub, :],
                                                     gate[:, sub, :])

                bdh = work.tile([P, 2 * P], F32R, tag="bdh", name="bdh")
                ps = psum.tile([P, 512], F32, tag="mmA", bufs=3, name="ps")
                psl = ps[:, :2 * P]
                nc.tensor.matmul(psl, _r(cw['bd_w1_k0']), xT[:, 0, cols],
                                 start=True, stop=False)
                nc.tensor.matmul(psl, _r(cw['bd_w1_k1']), xT[:, 1, cols],
                                 start=False, stop=False)
                nc.tensor.matmul(psl, _r(cw['bd_w1_k2']), xhi[0:1, cols],
                                 start=False, stop=True)
                nc.scalar.activation(bdh, psl, AF.Relu, bias=cw['bd_b1c'])

                bp_c = work.tile([P, 2, IN_DIM], F32, tag="bp_c", name="bp_c")
                for sub in range(2):
                    c = 2 * c2 + sub
                    rows = min(P, t_len - c * P)
                    ps = psum.tile([P, 512], F32, tag="mmB", bufs=3,
                                   name="ps")
                    psl = ps[:, :IN_DIM + 1]
                    nc.tensor.matmul(psl, _r(bdh[:, sub * P:(sub + 1) * P]),
                                     _r(cw['bd_w2p_k0']), start=True,
                                     stop=not flags['bd2_b'])
                    if flags['bd2_b']:
                        nc.tensor.matmul(psl, _r(ones_r[0:1, bass.ts(c, P)]),
                                         _r(cw['bd_w2p_b']), start=False,
                                         stop=True)
                    nc.scalar.activation(bp_c[:, sub, :], ps[:, :IN_DIM],
                                         AF.Exp, scale=-1.0)
                    nc.vector.tensor_scalar_add(bp_c[:, sub, :],
                                                bp_c[:, sub, :], 1.0)
                    nc.vector.reciprocal_approx_fast(bp_c[:, sub, :],
                                                     bp_c[:, sub, :])
                    nc.sync.dma_start(out=out1_d[s, c * P:c * P + rows, :],
                                      in_=bp_c[:rows, sub, :])
                st[c2] = (x_t, t1T, gate)

            def back(c2):
                x_t, t1T, gate = st.pop(c2)
                attn = work.tile([P, 2, IN_DIM], F32, tag="attn", name="attn")
                ssum = small.tile([P, 2], F32, tag="ssum", name="ssum")
                for sub in range(2):
                    scs = bass.ts(2 * c2 + sub, P)
                    ps = psum.tile([P, 512], F32, tag="mmB", bufs=3,
                                   name="ps")
                    psl = ps[:, :IN_DIM + 1]
                    for m in range(4):
                        nc.tensor.matmul(
                            psl, _r(t1T[:, m, sub * P:(sub + 1) * P]),
                            _r(cw['fa_w2p_k' + str(m)]), start=(m == 0),
                            stop=(m == 3 and not flags['fa2_b']))
                    if flags['fa2_b']:
                        nc.tensor.matmul(psl, _r(ones_r[0:1, scs]),
                                         _r(cw['fa_w2p_b']), start=False,
                                         stop=True)
                    nc.scalar.activation(attn[:, sub, :], ps[:, :IN_DIM],
                                         AF.Exp,
                                         accum_out=ssum[:, sub:sub + 1])
                nc.vector.reciprocal(ssum, ssum)
                for sub in range(2):
                    nc.vector.tensor_scalar(attn[:, sub, :], attn[:, sub, :],
                                            ssum[:, sub:sub + 1], None,
                                            op0=ALU.mult)
                ff = work.tile([P, 2, IN_DIM], F32, tag="ff", bufs=2,
                               name="ff")
                nc.vector.tensor_tensor(ff, x_t, attn, op=ALU.mult)
                nc.vector.tensor_tensor(ff, ff, gate, op=ALU.mult)

                ffT = work.tile([P, 2, 2 * P], F32R, tag="ffT", name="ffT")
                ffT_hi = work.tile([1, 2 * P], F32R, tag="ffT_hi",
                                   name="ffT_hi")
                for sub in range(2):
                    scol = bass.ts(sub, P)
                    ps_tr = psum.tile([P, 2, P], F32, tag="tr", bufs=2,
                                      name="ps_tr")
                    for blk in range(2):
                        nc.tensor.transpose(
                            ps_tr[:, blk, :],
                            ff[:, sub, blk * P:(blk + 1) * P], identity)
                    nc.vector.tensor_copy(out=ffT[:, :, scol], in_=ps_tr)
                    ps_t1 = psum.tile([P, 2, P], F32, tag="tr", bufs=2,
                                      name="ps_t1")
                    nc.tensor.transpose(ps_t1[0:1, 0, :], ff[:, sub, 256:257],
                                        identity)
                    nc.vector.tensor_copy(out=ffT_hi[:, scol],
                                          in_=ps_t1[0:1, 0, :])

                nfh = work.tile([P, 2 * P], F32R, tag="nfh", name="nfh")
                ps = psum.tile([P, 512], F32, tag="mmA", bufs=3, name="ps")
                psl = ps[:, :2 * P]
                nc.tensor.matmul(psl, _r(cw['nf_w1_k0']), ffT[:, 0, :],
                                 start=True, stop=False)
                nc.tensor.matmul(psl, _r(cw['nf_w1_k1']), ffT[:, 1, :],
                                 start=False, stop=False)
                nc.tensor.matmul(psl, _r(cw['nf_w1_k2']), ffT_hi,
                                 start=False, stop=True)
                nc.scalar.activation(nfh, psl, AF.Relu, bias=cw['nf_b1c'])

                r_t = work.tile([P, 2, IN_DIM], F32, tag="r_t", name="r_t")
                for sub in range(2):
                    ps = psum.tile([P, 512], F32, tag="mmB", bufs=3,
                                   name="ps")
                    psl = ps[:, :8]
                    nc.tensor.matmul(psl, _r(nfh[:, sub * P:(sub + 1) * P]),
                                     _r(cw['nf_w2p_k0']), start=True,
                                     stop=not flags['nf2_b'])
                    if flags['nf2_b']:
                        nc.tensor.matmul(
                            psl, _r(ones_r[0:1, bass.ts(2 * c2 + sub, P)]),
                            _r(cw['nf_w2p_b']), start=False, stop=True)
                    negc = small.tile([P, 4], F32, tag="negc", name="negc")
                    negk = small.tile([P, 4], F32, tag="negk", name="negk")
                    nc.scalar.activation(negc, psl[:, 0:4], AF.Identity,
                                         scale=-1.0)
                    nc.scalar.activation(negk, psl[:, 4:8], AF.Exp)
                    nc.scalar.activation(negk, negk, AF.Ln, bias=1.0)
                    nc.vector.tensor_tensor(negk, negk, negk, op=ALU.mult)
                    nc.vector.reciprocal(negk, negk)
                    nc.vector.tensor_scalar(negk, negk,
                                            -1.0 / (2.0 * 1.3 * 1.3), None,
                                            op0=ALU.mult)
                    for i in range(NF):
                        u_t = work.tile([P, IN_DIM], F32, tag="u_t", bufs=2,
                                        name="u_t")
                        nc.scalar.activation(u_t, fidx_b, AF.Square,
                                             bias=negc[:, i:i + 1])
                        nc.scalar.activation(u_t, u_t, AF.Exp,
                                             scale=negk[:, i:i + 1])
                        if i == 0:
                            nc.vector.tensor_scalar(r_t[:, sub, :], u_t, 1.0,
                                                    None, op0=ALU.subtract)
                        else:
                            nc.vector.scalar_tensor_tensor(
                                r_t[:, sub, :], u_t, 1.0, r_t[:, sub, :],
                                op0=ALU.subtract, op1=ALU.mult)
                nc.gpsimd.memset(r_t[:, :, 0:n_prot], 1.0)
                nc.vector.tensor_tensor(ff, ff, r_t, op=ALU.mult)
                st[('ff', c2)] = ff

            def ffn_tail(c2):
                ff = st.pop(('ff', c2))
                for sub in range(2):
                    c = 2 * c2 + sub
                    ccols = bass.ts(c, P)
                    ps_tr = psum.tile([P, 2, P], F32, tag="tr", bufs=2,
                                      name="ps_tr")
                    for blk in range(2):
                        nc.tensor.transpose(
                            ps_tr[:, blk, :],
                            ff[:, sub, blk * P:(blk + 1) * P], identity)
                    nc.vector.tensor_copy(out=fT[:, 0:2, ccols], in_=ps_tr)
                    ps_t1 = psum.tile([P, 2, P], F32, tag="tr", bufs=2,
                                      name="ps_t1")
                    nc.tensor.transpose(ps_t1[0:1, 0, :], ff[:, sub, 256:257],
                                        identity)
                    nc.vector.tensor_copy(out=ffnhi[0:1, ccols],
                                          in_=ps_t1[0:1, 0, :])

            front(0)
            back(0)
            for c2 in range(1, nc2):
                front(c2)          # PE filler while notch(c2-1) completes
                ffn_tail(c2 - 1)
                back(c2)
            ffn_tail(nc2 - 1)

            # ================= DFSMN layers =================
            # table state: sqrt_and_friends (Sqrt/Identity/Copy/Relu/Square)
            for li in range(4):
                dout = HID[li]
                mt = dout // P
                for c in range(nch):
                    ccols = bass.ts(c, P)
                    ps = psum.tile([P, 512], F32, tag="mmA", bufs=3)
                    psl = ps[:, :dout]
                    if li == 0:
                        nc.tensor.matmul(psl, fT[:, 0, ccols],
                                         _r(cw['lw0_k0']), start=True,
                                         stop=False)
                        nc.tensor.matmul(psl, fT[:, 1, ccols],
                                         _r(cw['lw0_k1']), start=False,
                                         stop=False)
                        nc.tensor.matmul(psl, _r(ffnhi[:, ccols]),
                                         _r(cw['lw0_k2']), start=False,
                                         stop=True)
                    else:
                        nk = HID[li - 1] // P
                        bias_f = flags[f'l{li}_b']
                        for k in range(nk):
                            nc.tensor.matmul(
                                psl, fT[:, k, ccols],
                                _r(cw[f'lw{li}_k{k}']), start=(k == 0),
                                stop=(k == nk - 1 and not bias_f))
                        if bias_f:
                            nc.tensor.matmul(psl, _r(ones_r[0:1, ccols]),
                                             _r(cw[f'lw{li}_b']), start=False,
                                             stop=True)
                    rows_c = min(P, t_len - c * P)
                    if rows_c < P:
                        zb = (rows_c // 32) * 32
                        nc.vector.memset(h_t[zb:P, c, :dout].bitcast(F32), 0.0)
                    nc.scalar.copy(out=h_t[:rows_c, c, :dout],
                                   in_=psl[:rows_c, :])

                for c in range(nch):
                    ccols = bass.ts(c, P)
                    ps_c = psum.tile([P, 512], F32, tag="mmB", bufs=3)
                    pcl = ps_c[:, :dout]
                    sblk = cw[f'bself0_{li}'] if c == 0 else cw[f'bself_{li}']
                    last = (c == nch - 1)
                    nc.tensor.matmul(pcl, _r(sblk), h_t[:, c, :dout],
                                     start=True, stop=(c == 0 and last))
                    if c > 0:
                        nc.tensor.matmul(pcl, _r(cw[f'bprev_{li}']),
                                         h_t[:, c - 1, :dout],
                                         start=False, stop=last)
                    if not last:
                        nc.tensor.matmul(pcl, _r(cw[f'bnext_{li}']),
                                         h_t[:, c + 1, :dout],
                                         start=False, stop=True)
                    st6 = small.tile([P, 6], F32, tag="st6")
                    mv = small.tile([P, 2], F32, tag="mv")
                    nc.vector.bn_stats(st6, pcl)
                    nc.vector.bn_aggr(mv, st6)
                    rstd = small.tile([P, 1], F32, tag="rstd")
                    nmr = small.tile([P, 1], F32, tag="nmr")
                    nc.scalar.activation(rstd, mv[:, 1:2], AF.Sqrt, bias=eps_t)
                    nc.vector.reciprocal(rstd, rstd)
                    nc.vector.scalar_tensor_tensor(nmr, mv[:, 0:1], -1.0, rstd,
                                                   op0=ALU.mult, op1=ALU.mult)
                    ps_s = psum.tile([P, 512], F32, tag="mmA", bufs=3)
                    pss = ps_s[:, :dout]
                    nc.tensor.matmul(pss, xT[:, 0, ccols],
                                     _r(cw[f'sw{li}_k0']), start=True,
                                     stop=False)
                    nc.tensor.matmul(pss, xT[:, 1, ccols],
                                     _r(cw[f'sw{li}_k1']), start=False,
                                     stop=False)
                    nc.tensor.matmul(pss, _r(xhi[:, ccols]),
                                     _r(cw[f'sw{li}_k2']), start=False,
                                     stop=True)
                    ln_t = work.tile([P, 512], F32, tag="ln_t", bufs=2)
                    lnl = ln_t[:, :dout]
                    nc.scalar.activation(lnl, pcl, AF.Identity, scale=rstd,
                                         bias=nmr)
                    if has_g:
                        nc.vector.tensor_tensor(lnl, lnl, cw[f'g_bcast{li}'],
                                                op=ALU.mult)
                    nc.vector.tensor_tensor(lnl, lnl, pss, op=ALU.add)
                    if li < 3:
                        nc.scalar.activation(lnl, lnl, AF.Relu)
                        ps_tr = psum.tile([P, mt, P], F32, tag="tr", bufs=2,
                                          name="ps_tr")
                        for m in range(mt):
                            nc.tensor.transpose(
                                ps_tr[:, m, :], lnl[:, m * P:(m + 1) * P],
                                identity)
                        nc.vector.tensor_copy(out=fT[:, :mt, ccols],
                                              in_=ps_tr)
                    else:
                        nc.scalar.activation(f3a[:, c, :], lnl, AF.Relu)

            # ================= output stage =================
            for c in range(nch):
                ccols = bass.ts(c, P)
                rows = min(P, t_len - c * P)
                ps_tr = psum.tile([P, 2, P], F32, tag="tr", bufs=2)
                nc.tensor.transpose(ps_tr[:, 0, :], f3a[:, c, :], identity)
                f3T = work.tile([P, P], F32R, tag="f3T", bufs=2)
                nc.vector.tensor_copy(out=f3T, in_=ps_tr[:, 0, :])

                ps_o = psum.tile([P, 512], F32, tag="mmA", bufs=3)
                pso = ps_o[:, :IN_DIM + 1]
                nc.tensor.matmul(pso, f3T, _r(cw['outwp_k0']),
                                 start=True, stop=not flags['out_b'])
                if flags['out_b']:
                    nc.tensor.matmul(pso, _r(ones_r[0:1, ccols]),
                                     _r(cw['outwp_b']), start=False,
                                     stop=True)
                ps_g = psum.tile([P, 512], F32, tag="mmB", bufs=3)
                psg = ps_g[:, :IN_DIM + 1]
                nc.tensor.matmul(psg, f3T, _r(cw['gatewp_k0']),
                                 start=True, stop=not flags['gate_b'])
                if flags['gate_b']:
                    nc.tensor.matmul(psg, _r(ones_r[0:1, ccols]),
                                     _r(cw['gatewp_b']), start=False,
                                     stop=True)
                og = work.tile([P, IN_DIM], F32, tag="og", bufs=2)
                nc.scalar.activation(og, ps_g[:, :IN_DIM], AF.Exp, scale=-1.0)
                nc.vector.tensor_scalar_add(og, og, 1.0)
                nc.vector.reciprocal_approx_fast(og, og)
                xb = work.tile([P, IN_DIM], F32, tag="xb", bufs=2)
                bpb = work.tile([P, IN_DIM], F32, tag="bpb", bufs=2)
                nc.sync.dma_start(out=xb[:rows, :],
                                  in_=x_d[s, c * P:c * P + rows, :])
                nc.sync.dma_start(out=bpb[:rows, :],
                                  in_=out1_d[s, c * P:c * P + rows, :])
                bm = work.tile([P, IN_DIM], mybir.dt.uint8, tag="bm", bufs=2)
                nc.vector.tensor_scalar(bm[:rows, :], bpb[:rows, :],
                                        0.6, None, op0=ALU.is_gt)
                res = work.tile([P, IN_DIM], F32, tag="res", bufs=2)
                nc.vector.tensor_copy(out=res[:rows, :],
                                      in_=ps_o[:rows, :IN_DIM])
                nc.vector.copy_predicated(res[:rows, :], bm[:rows, :],
                                          xb[:rows, :])
                nc.vector.tensor_tensor(res[:rows, :], res[:rows, :],
                                        og[:rows, :], op=ALU.mult)
                nc.sync.dma_start(out=out0_d[s, c * P:c * P + rows, :],
                                  in_=res[:rows, :])
    nc.compile()
    return nc


# ----------------------------------------------------------------------------
# entry point
# ----------------------------------------------------------------------------

def _build_cached(n_seq, t_len, flags):
    key = (n_seq, t_len, tuple(sorted(flags.items())))
    if key not in _RUN_CACHE:
        _RUN_CACHE[key] = build_program(n_seq, t_len, flags)
    return _RUN_CACHE[key]


def kernel(x, params, **run_kwargs):
    from concourse.bass_utils import run_bass_kernel_spmd

    x = np.ascontiguousarray(np.asarray(x, np.float32))
    host = _prep_host(params)
    flags = host['flags']
    nc = _build_cached(B_CORE, T, flags)

    w_map = {k: np.ascontiguousarray(v) for k, v in host.items()
             if isinstance(v, np.ndarray)}
    in_maps = []
    for core in range(N_CORES):
        m = dict(w_map)
        m['x'] = x[core * B_CORE:(core + 1) * B_CORE]
        in_maps.append(m)

    res = run_bass_kernel_spmd(nc, in_maps, core_ids=list(range(N_CORES)),
                               **run_kwargs)
    out0 = np.concatenate([r['out0'] for r in res.results], 0)
    out1 = np.concatenate([r['out1'] for r in res.results], 0)
    return out0, out1
